# revision 41
# baseline (speedup 1.0000x reference)
"""Trainium2 Bass kernel for nn_ConvAConnect (per-sample-weight 3x3 conv).

Pure data parallel: 16 samples per core on 8 cores. Per (sample, kh) the
conv maps to PE matmuls via a block-Toeplitz weight matrix (lhsT [K=128:
16 in-pixels x 8 cin, M=112: 14 out-pixels x 8 cout]); the input ships
host-transposed to [(x*8+ci), y] strip layout in bf16, the output is
written in [(xo*8+co), strip, y] bf16 and un-permuted/cast on the host.
Bias (bias*Berr) is fused into the PSUM->SBUF copy on the ACT/DVE engines.

Measured 117.7us (8-core SPMD, core-0 profile; ambient machine
variance up to +-20us under neighbor HBM contention) vs the 122.6us
baseline:
- 18 main strips (xo 0..251) + a packed tail: 2 samples per matmul
  (K=2x48 rows, M=2x32) for xo 252..255 instead of a 16th full strip
  (which spent 16 full 256-column PE passes on 4/14 useful columns).
  Saves ~2.5us PE and ~1MB/core HBM.
- Per-sample xs DMAs (1.19MB, one 9.3KB contiguous run per partition);
  tw pair-packed and the tail rhs|lhsT merged into one [96,450] buffer
  to amortize the ~0.6-0.8us per-DMA completion overhead.
- Outputs split across the two HWDGE queues (group A -> sync, group B
  -> scalar/qAct) so the drain runs on two queues in parallel; the
  final sample's outputs ship fine-grained across BOTH queues (A in 2
  halves, B per-unit), each piece leaving as its own bias-add
  finishes -- both queues end within 0.1us and the drain is ~3.9us
  after the last matmul.
- 9 dummy matmuls bridge the PE HAM clock gate through the DMA
  lead-in; sample 0's xs ships in 4 ascending chunks with kh-inner
  matmul order so real PE work starts at ~11.6us (vs 14.1 baseline).

Known-bad (all measured slower): SBUF->SBUF x-halo reconstruction on a
third queue (512B-run packets starve the main streams, +14us);
shipping xs without the y-pad columns (breaks run contiguity -> input
queue drops to 190GB/s, +15us); pair-packed 2.4MB xs DMAs (bursty PE
starvation re-triggers the HAM throttle, +12us); routing sample-0
inputs over the sync queue (first data still lands ~9us - completion
latency dominates - and the PE starves to 23us, +12us).
"""

import os
import sys

import numpy as np

for _p in ("/opt/trn_rl_repo", "/root/.axon_site/_ro/trn_rl_repo"):
    if os.path.isdir(_p) and _p not in sys.path:
        sys.path.insert(0, _p)

B, H, W_IMG, CIN = 128, 256, 256, 8
KH, KW, COUT = 3, 3, 8

NCORES = 8
BPC = B // NCORES
NPIX = 14
SW = NPIX * COUT
NSM = 18            # main strips, xo 0..251
Y = H
YP = Y + 2
KP = 128
NP2 = BPC // 2      # sample-pairs per core
TK = 96             # tail contraction rows (2 samples x 6 xi x 8 ci)
TM = 64             # tail out rows (2 samples x 4 xo x 8 co)
TTW = YP + KH * TM  # merged tail buffer width (rhs | lhsT)

TRACE = False
LAST_RESULT = [None]

_NC_CACHE = [None]


def _build_nc():
    import concourse.bass as bass
    import concourse.mybir as mybir
    from concourse.tile import TileContext

    f32 = mybir.dt.float32
    bf16 = mybir.dt.bfloat16
    nc = bass.Bass()
    xs = nc.declare_dram_parameter("xs", [BPC, KP, NSM, YP], bf16,
                                   isOutput=False)
    tw = nc.declare_dram_parameter("tw", [NP2, KP, 2, KH, SW], bf16,
                                   isOutput=False)
    tt = nc.declare_dram_parameter("tt", [NP2, TK, TTW], bf16, isOutput=False)
    bi = nc.declare_dram_parameter("bi", [SW, BPC], f32, isOutput=False)
    bit = nc.declare_dram_parameter("bit", [TM, NP2], f32, isOutput=False)
    zt = nc.declare_dram_parameter("zt", [BPC, SW, NSM, Y], bf16, isOutput=True)
    ztt = nc.declare_dram_parameter("ztt", [NP2, TM, Y], bf16, isOutput=True)

    # 9 two-strip units; group A = strips 0..9, group B = strips 10..17
    units = [(s, 2) for s in range(0, NSM, 2)]
    groups = [units[:5], units[5:]]

    with TileContext(nc) as tc:
        with (
            tc.tile_pool(name="xp", bufs=4) as xp,
            tc.tile_pool(name="wp", bufs=2) as wp,
            tc.tile_pool(name="op", bufs=4) as op,
            tc.tile_pool(name="bp", bufs=1) as bp,
            tc.tile_pool(name="tp", bufs=2) as tpp,
            tc.tile_pool(name="pp", bufs=8, space="PSUM") as pp,
        ):
            bias_t = bp.tile([SW, BPC], f32, tag="bias")
            nc.sync.dma_start(out=bias_t, in_=bi[:, :])
            bias_tt = bp.tile([TM, NP2], f32, tag="biast")
            nc.sync.dma_start(out=bias_tt, in_=bit[:, :])

            # PE warm-up: junk matmuls during the DMA lead-in so the HAM
            # clock gate flips to 2.4GHz; sized to end right when the
            # first strips arrive so the PE never idles in between.
            junk = bp.tile([TK, 576], bf16, tag="junk")
            nc.vector.memset(junk, 0.0)
            pj = pp.tile([128, 2 * Y], f32, name="pt", tag="pt")
            NDUMMY = 9
            for i in range(NDUMMY):
                nc.tensor.matmul(
                    out=pj[0:TM, 0:Y],
                    lhsT=junk[0:TK, 0:TM],
                    rhs=junk[0:TK, 320:320 + Y],
                    start=(i == 0),
                    stop=(i == NDUMMY - 1),
                )

            for b in range(BPC):
                par = b % 2
                g = b // 2
                xtile = xp.tile([KP, NSM, YP], bf16)
                if par == 0:
                    wtile = wp.tile([KP, 2, KH, SW], bf16)
                    nc.gpsimd.dma_start(out=wtile, in_=tw[g])
                if b == 0:
                    for lo, hi in ((0, 2), (2, 6), (6, 10), (10, NSM)):
                        nc.gpsimd.dma_start(out=xtile[:, lo:hi, :],
                                            in_=xs[b, :, lo:hi, :])
                else:
                    nc.gpsimd.dma_start(out=xtile, in_=xs[b])
                if par == 0:
                    # prefetch the pair's tail inputs on the input queue
                    ttile = tpp.tile([TK, TTW], bf16, tag="tt")
                    nc.gpsimd.dma_start(out=ttile, in_=tt[g])
                else:
                    # tail for this pair: xo 252..255, both samples in one
                    # matmul; runs early so ztt never gates the kernel end
                    ptt = pp.tile([128, 2 * Y], f32, name="pt", tag="pt")
                    for kh in range(KH):
                        nc.tensor.matmul(
                            out=ptt[0:TM, 0:Y],
                            lhsT=ttile[:, YP + TM * kh:YP + TM * (kh + 1)],
                            rhs=ttile[:, kh:kh + Y],
                            start=(kh == 0),
                            stop=(kh == KH - 1),
                        )
                    ot2 = tpp.tile([TM, Y], bf16, tag="ot2")
                    if g % 2 == 0:
                        nc.vector.tensor_scalar_add(
                            out=ot2, in0=ptt[0:TM, 0:Y],
                            scalar1=bias_tt[:, g:g + 1],
                        )
                    else:
                        nc.scalar.add(
                            out=ot2, in_=ptt[0:TM, 0:Y],
                            add=bias_tt[:, g:g + 1],
                        )
                    nc.sync.dma_start(out=ztt[g], in_=ot2)

                nadd = 0
                for gi, grp in enumerate(groups):
                    g0 = grp[0][0]
                    nstrips = sum(w for _, w in grp)
                    pts = [pp.tile([128, 2 * Y], f32, name="pt", tag="pt")
                           for _ in grp]
                    otile = op.tile([SW, nstrips * Y], bf16,
                                    tag=f"ot{nstrips}")
                    if b == 0 and gi == 0:
                        # kh-inner so each unit finishes as its strips
                        # arrive from the split first-sample DMAs
                        mm_order = [(kh, j) for j in range(len(grp))
                                    for kh in range(KH)]
                    else:
                        mm_order = [(kh, j) for kh in range(KH)
                                    for j in range(len(grp))]
                    for kh, j in mm_order:
                        s, w = grp[j]
                        nc.tensor.matmul(
                            out=pts[j][0:SW, 0:w * Y],
                            lhsT=wtile[:, par, kh, :],
                            rhs=xtile[:, s:s + w, kh:kh + Y],
                            start=(kh == 0),
                            stop=(kh == KH - 1),
                        )
                    for j, (s, w) in enumerate(grp):
                        c0 = (s - g0) * Y
                        if nadd % 2 == 0:
                            nc.vector.tensor_scalar_add(
                                out=otile[0:SW, c0:c0 + w * Y],
                                in0=pts[j][0:SW, 0:w * Y],
                                scalar1=bias_t[:, b:b + 1],
                            )
                        else:
                            nc.scalar.add(
                                out=otile[0:SW, c0:c0 + w * Y],
                                in_=pts[j][0:SW, 0:w * Y],
                                add=bias_t[:, b:b + 1],
                            )
                        nadd += 1
                    # group A -> sync queue, group B -> scalar queue; the
                    # second-to-last sample swaps, and the last sample
                    # splits each group across BOTH queues at fine grain
                    # so the final drain runs fully parallel and each
                    # piece leaves as soon as its own bias-add finishes
                    if b == BPC - 1:
                        if gi == 0:
                            half = nstrips // 2
                            nc.sync.dma_start(
                                out=zt[b, :, g0:g0 + half, :],
                                in_=otile[0:SW, 0:half * Y],
                            )
                            nc.scalar.dma_start(
                                out=zt[b, :, g0 + half:g0 + nstrips, :],
                                in_=otile[0:SW, half * Y:nstrips * Y],
                            )
                        else:
                            for k, (s, w) in enumerate(grp):
                                eng = nc.sync if k % 2 == 0 else nc.scalar
                                c0 = (s - g0) * Y
                                eng.dma_start(
                                    out=zt[b, :, s:s + w, :],
                                    in_=otile[0:SW, c0:c0 + w * Y],
                                )
                    elif (gi == 0) ^ (b == BPC - 2):
                        nc.sync.dma_start(
                            out=zt[b, :, g0:g0 + nstrips, :],
                            in_=otile[0:SW, 0:nstrips * Y],
                        )
                    else:
                        nc.scalar.dma_start(
                            out=zt[b, :, g0:g0 + nstrips, :],
                            in_=otile[0:SW, 0:nstrips * Y],
                        )
    _split_multi_waits(nc, mybir)
    return nc


def _split_multi_waits(nc, mybir):
    nid = [0]
    for fn in nc.m.functions:
        for blk in fn.blocks:
            out = []
            for inst in blk.instructions:
                si = inst.sync_info
                if si is not None and si.on_wait and len(si.on_wait) > 1:
                    waits = list(si.on_wait)
                    for w in waits[:-1]:
                        nid[0] += 1
                        out.append(mybir.InstNoOp(
                            name=f"nopw-{nid[0]}",
                            engine=inst.engine,
                            ins=[],
                            outs=[],
                            sync_info=mybir.SyncInfo(on_wait=[w], on_update=[]),
                        ))
                    inst.sync_info = mybir.SyncInfo(
                        on_wait=[waits[-1]],
                        on_update=list(si.on_update or []),
                    )
                out.append(inst)
            blk.instructions[:] = out


def _get_nc():
    if _NC_CACHE[0] is None:
        _NC_CACHE[0] = _build_nc()
    return _NC_CACHE[0]


def host_prep(X, W, bias, Werr, Berr):
    X = np.asarray(X, np.float32)
    W = np.asarray(W, np.float32)
    bias = np.asarray(bias, np.float32)
    Werr = np.asarray(Werr, np.float32)
    Berr = np.asarray(Berr, np.float32)

    memW = W[None] * Werr                      # [B, kh, kw, cin, cout]
    TW = np.zeros((B, KP, KH, SW), np.float32)
    for kw in range(KW):
        blk = memW[:, :, kw].transpose(0, 2, 1, 3)   # [B, ci, kh, co]
        for xo in range(NPIX):
            xi = xo + kw
            TW[:, xi * 8:(xi + 1) * 8, :, xo * 8:(xo + 1) * 8] = blk

    # tail lhsT: block-diag over the 2 samples of each pair
    TWT = np.zeros((B // 2, TK, KH, TM), np.float32)
    for j in range(2):
        blkj = (memW[j::2]).transpose(0, 1, 3, 4, 2)  # [B/2, kh, ci, co, kw]
        for kw in range(KW):
            blk = blkj[..., kw].transpose(0, 2, 1, 3)  # [B/2, ci, kh, co]
            for xo in range(4):
                xi = xo + kw
                TWT[:, 48 * j + 8 * xi:48 * j + 8 * (xi + 1), :,
                    32 * j + 8 * xo:32 * j + 8 * (xo + 1)] = blk

    BIT = np.tile(bias[None] * Berr, (1, NPIX))        # [B, SW]
    membias = bias[None] * Berr                        # [B, COUT]
    BIT2 = np.zeros((B // 2, TM), np.float32)
    for j in range(2):
        BIT2[:, 32 * j:32 * (j + 1)] = np.tile(membias[j::2], (1, 4))

    import ml_dtypes
    bf16 = ml_dtypes.bfloat16
    XTP = np.zeros((B, 2176, YP), bf16)
    XTP[:, 8:8 + W_IMG * CIN, 1:1 + Y] = \
        X.transpose(0, 2, 3, 1).reshape(B, W_IMG * CIN, Y)
    # main strips: [B, 128, 18, YP]
    XS = np.empty((B, KP, NSM, YP), bf16)
    for s in range(NSM):
        XS[:, :, s, :] = XTP[:, 112 * s:112 * s + KP, :]
    # tw pair-packed: [B/2, 128, 2, KH, SW]
    TWP = np.empty((B // 2, KP, 2, KH, SW), bf16)
    for j in range(2):
        TWP[:, :, j] = TW[j::2]
    # merged tail buffer: rhs rows (xi 251..256 of both samples) | lhsT
    TT = np.zeros((B // 2, TK, TTW), bf16)
    TT[:, :, :YP] = XTP[:, 2016:2064].reshape(B // 2, 2 * 48, YP)
    for kh in range(KH):
        TT[:, :, YP + TM * kh:YP + TM * (kh + 1)] = TWT[:, :, kh, :]
    return XS, TWP, TT, BIT, BIT2


def host_unpack(zt_all, ztt_all):
    out = np.empty((B, Y, W_IMG, COUT), np.float32)
    z = zt_all.reshape(B, NPIX, COUT, NSM, Y)
    z = z.transpose(0, 4, 3, 1, 2).reshape(B, Y, NSM * NPIX, COUT)
    out[:, :, :NSM * NPIX, :] = z
    zt2 = ztt_all.reshape(B // 2, 2, 4, COUT, Y)   # [pair, j, xo, co, y]
    for j in range(2):
        out[j::2, :, NSM * NPIX:, :] = zt2[:, j].transpose(0, 3, 1, 2)
    return out


def kernel(X, W, bias, Werr, Berr):
    from concourse.bass_utils import run_bass_kernel_spmd

    XS, TWP, TT, BIT, BIT2 = host_prep(X, W, bias, Werr, Berr)
    in_maps = []
    for m in range(NCORES):
        sl = slice(m * BPC, (m + 1) * BPC)
        sl2 = slice(m * NP2, (m + 1) * NP2)
        in_maps.append({
            "xs": np.ascontiguousarray(XS[sl]),
            "tw": np.ascontiguousarray(TWP[sl2]),
            "tt": np.ascontiguousarray(TT[sl2]),
            "bi": np.ascontiguousarray(BIT[sl].T),
            "bit": np.ascontiguousarray(BIT2[sl2].T),
        })
    nc = _get_nc()
    res = run_bass_kernel_spmd(nc, in_maps, core_ids=list(range(NCORES)), trace=TRACE)
    LAST_RESULT[0] = res
    zt_all = np.concatenate([r["zt"] for r in res.results], axis=0)
    ztt_all = np.concatenate([r["ztt"] for r in res.results], axis=0)
    return host_unpack(zt_all, ztt_all)
